# revision 1
# baseline (speedup 1.0000x reference)
"""Trainium2 Bass kernel for nn_ExpSelfAttention (dense transformer block).

Math (per batch item b, all f32 data):
    y  = LN(x; g1, beta1);  z = y @ w_lin.T + b_lin
    attn = W @ z            (W = causal exp-decay matrix, alpha=0.9)
    x2 = x + attn
    y2 = LN(x2; g2, beta2); h = relu(y2 @ w1.T + b1)
    out = x2 + h @ w2.T + b2

Sharding: data parallel over batch (16 / 8 cores = 2 per core); weights and
the (input-independent) decay-matrix blocks replicated. No collectives.

Kernel strategy per core:
  - LN gains folded into the (pre-transposed) weights, LN betas into bias
    vectors; biases applied on PSUM eviction (per-partition ACT bias where
    the layout allows, broadcast tiles + DVE adds otherwise).
  - The S x S decay matmul is block-banded: with alpha=0.9, W's off-diagonal
    128-blocks decay by alpha^128 ~ 1.4e-6 per lag, so W @ z reduces exactly
    (to f32 resolution) to a block-diagonal matmul + NLAG lag-block matmuls
    whose matrices are shared across blocks. O(S*T*B*D) instead of
    O(S^2*B*D), with no serial carry chain.
  - Matmuls run in float32r (1 cyc/row at N>=256 vs 4 for plain f32;
    ~1.5e-4 relative rounding) accumulating in fp32 PSUM. Every f32r matmul
    input is produced by a rounding writer (DVE/ACT ops or gpsimd casting
    DMA) to satisfy the BIR verifier.
  - Software pipelined in 512-token steps: step i+1's load/LN1/transpose and
    projection matmuls are emitted between step i's mixing and FFN so the
    TensorEngine always has front-end work while LN chains run on DVE/ACT.
  - activations transposed on the PE (f32r transpose mode) since the
    contraction dim must sit on partitions for both matmul operands.
"""

import sys
from contextlib import ExitStack

for _p in ("/opt/trn_rl_repo", "/opt/pypackages"):
    if _p not in sys.path:
        sys.path.insert(0, _p)

import numpy as np

import concourse.bass as bass
import concourse.mybir as mybir
import concourse.tile as tile
from concourse import bacc
from concourse.bass_utils import run_bass_kernel_spmd
from concourse.masks import make_identity

ALPHA, EPS = 0.9, 1e-5
S, B, D, FF = 2048, 16, 512, 2048
NCORES = 8
BL = B // NCORES            # batch items per core
T = 128                     # mixing block
CB = 4                      # blocks per chunk
NBLK = S // T               # 16
NCHUNK = NBLK // CB         # 4
HC = 256                    # FFN half-chunk tokens
NFT = FF // 128             # 16 f-tiles
KD = D // 128               # 4 d-tiles
NLAG = 1                    # decay lag blocks kept (lag>=2 < 2e-12 relative)

F32 = mybir.dt.float32
F32R = mybir.dt.float32r
USE_F32R = True
MMDT = F32R if USE_F32R else F32


AF = mybir.ActivationFunctionType


def _host_consts():
    """Decay-matrix derived constants, f64 -> f32 (mirrors reference)."""
    i = np.arange(S, dtype=np.float64)
    diff = i[:, None] - i[None, :]
    with np.errstate(under="ignore"):
        W = np.where(diff >= 0, ALPHA ** (diff + 1), 0.0)
        W = W + np.diag(1.0 - W.sum(axis=1))
        W = W.astype(np.float32)
        # per-block transposed diag-blocks (lhsT of the local mixing matmul)
        blocks = [
            np.ascontiguousarray(W[c * T : (c + 1) * T, c * T : (c + 1) * T].T)
            for c in range(NBLK)
        ]
        # dedupe identical blocks (diag correction saturates after ~block 1)
        uniq, idx = [], []
        for blk in blocks:
            for j, u in enumerate(uniq):
                if np.array_equal(blk, u):
                    idx.append(j)
                    break
            else:
                idx.append(len(uniq))
                uniq.append(blk)
        wblkT = np.stack(uniq)  # [NU, T, T]
        # lag matrices: W[i0:i0+T, i0-l*T:i0-(l-1)*T] is constant across i0
        lags = []
        for l in range(1, NLAG + 1):
            L = W[l * T : (l + 1) * T, 0:T]
            for i0 in range(l * T, S, T):
                assert np.array_equal(W[i0 : i0 + T, i0 - l * T : i0 - (l - 1) * T], L)
            lags.append(np.ascontiguousarray(L.T))
        wlagT = np.stack(lags)  # [NLAG, T, T]
    return wblkT.astype(np.float32), idx, wlagT.astype(np.float32)


_WBLKT, _BLKIDX, _WLAGT = _host_consts()
NU = _WBLKT.shape[0]

_NC_CACHE = {}


def build_nc():
    key = MMDT
    if key in _NC_CACHE:
        return _NC_CACHE[key]
    nc = bacc.Bacc()

    x_d = nc.declare_dram_parameter("x", [S, BL, D], F32, isOutput=False)
    wp_d = nc.declare_dram_parameter("wp", [D, D], F32, isOutput=False)
    zb_d = nc.declare_dram_parameter("zb", [D], F32, isOutput=False)
    w1t_d = nc.declare_dram_parameter("w1t", [D, FF], F32, isOutput=False)
    hb_d = nc.declare_dram_parameter("hb", [FF], F32, isOutput=False)
    w2t_d = nc.declare_dram_parameter("w2t", [FF, D], F32, isOutput=False)
    b2_d = nc.declare_dram_parameter("b2", [D], F32, isOutput=False)
    wblk_d = nc.declare_dram_parameter("wblk", [NU, T, T], F32, isOutput=False)
    wlag_d = nc.declare_dram_parameter("wlag", [NLAG, T, T], F32, isOutput=False)
    out_d = nc.declare_dram_parameter("out", [S, BL, D], F32, isOutput=True)

    with tile.TileContext(nc) as tc, ExitStack() as ctx:
            pool = lambda name, bufs, **kw: ctx.enter_context(
                tc.tile_pool(name=name, bufs=bufs, **kw)
            )
            wgt = pool("wgt", 1)
            stage = pool("stage", 1)
            xin = pool("xin", 9)
            lnp = pool("ln", 6)
            yppp = pool("ypp", 2)
            xtp = pool("xt", 6)
            y2tp = pool("y2t", 2)
            zp = pool("z", 10)
            x2p = pool("x2", 5)
            hp = pool("h", 2)
            outp = pool("outp", 3)
            psmm = pool("psmm", 5, space="PSUM")
            pstr = pool("pstr", 3, space="PSUM")
            # ---------------- one-time setup ----------------

            xpre = {}

            def preload_x(i):
                b, c = steps[i]
                tiles = []
                for t in range(CB):
                    s0 = (c * CB + t) * T
                    xt = xin.tile([128, D], F32, tag="x")
                    nc.sync.dma_start(xt[:], x_d.ap()[s0 : s0 + T, b, :])
                    tiles.append(xt)
                xpre[i] = tiles

            steps = [(b, c) for b in range(BL) for c in range(NCHUNK)]
            preload_x(0)
            ident_f = stage.tile([128, 128], F32, tag="ident_f")
            make_identity(nc, ident_f[:])
            ident = wgt.tile([128, 128], MMDT, tag="ident")
            nc.vector.tensor_copy(ident[:], ident_f[:])
            eps_t = wgt.tile([128, 1], F32, tag="eps")
            nc.vector.memset(eps_t[:], EPS)
            neg1_t = wgt.tile([128, 1], F32, tag="neg1")
            nc.vector.memset(neg1_t[:], -1.0)
            zb_bc = wgt.tile([128, D], F32, tag="zb")
            nc.sync.dma_start(
                zb_bc[:],
                bass.AP(tensor=zb_d, offset=0, ap=[[0, 128], [1, D]]),
            )
            b2_bc = wgt.tile([128, D], F32, tag="b2")
            nc.sync.dma_start(
                b2_bc[:],
                bass.AP(tensor=b2_d, offset=0, ap=[[0, 128], [1, D]]),
            )
            hb_sb = wgt.tile([128, NFT], F32, tag="hb")
            nc.sync.dma_start(
                hb_sb[:],
                bass.AP(tensor=hb_d, offset=0, ap=[[1, 128], [128, NFT]]),
            )

            def load_round(dram_ap, shape, tag):
                """Casting DMA (gpsimd SWDGE) f32 DRAM -> resident f32r tile."""
                rt = wgt.tile(shape, MMDT, tag=tag)
                nc.gpsimd.dma_start(rt[:], dram_ap)
                return rt

            wp_r = load_round(
                wp_d.ap().rearrange("(kd p) e -> p kd e", p=128), [128, KD, D], "wp"
            )
            wblk_r = load_round(
                wblk_d.ap().rearrange("b j r -> j b r"), [128, NU, T], "wblk"
            )
            wlag_r = load_round(
                wlag_d.ap().rearrange("b j r -> j b r"), [128, NLAG, T], "wlag"
            )

            # ---------------- helpers ----------------
            def layer_norm_stats(xt):
                """-> (mean, rstd) [128,1] tiles."""
                st = lnp.tile([128, 6], F32, tag="bnst")
                nc.vector.bn_stats(st[:], xt)
                mv = lnp.tile([128, 2], F32, tag="bnmv")
                nc.vector.bn_aggr(mv[:], st[:])
                rstd = lnp.tile([128, 1], F32, tag="rstd")
                nc.scalar.activation(
                    rstd[:], mv[:, 1:2], AF.Sqrt, bias=eps_t[:], scale=1.0
                )
                nc.vector.reciprocal(rstd[:], rstd[:])
                return mv, rstd

            def normalize_transpose(xt, tag, dest, dest_off, use_dve=False, stats=None):
                """LN(xt) -> transposed [d, s] written into dest[:, :, off:off+128]."""
                mv, rstd = stats if stats is not None else layer_norm_stats(xt)
                ypp = yppp.tile([128, D], MMDT, tag=tag)
                if use_dve:
                    nc.vector.tensor_scalar(
                        out=ypp[:],
                        in0=xt,
                        scalar1=mv[:, 0:1],
                        scalar2=rstd[:],
                        op0=mybir.AluOpType.subtract,
                        op1=mybir.AluOpType.mult,
                    )
                else:
                    nbias = lnp.tile([128, 1], F32, tag="nbias")
                    nc.vector.tensor_scalar(
                        out=nbias[:],
                        in0=rstd[:],
                        scalar1=mv[:, 0:1],
                        scalar2=neg1_t[:],
                        op0=mybir.AluOpType.mult,
                        op1=mybir.AluOpType.mult,
                    )
                    nc.scalar.activation(
                        ypp[:], xt, AF.Identity, bias=nbias[:], scale=rstd[:]
                    )
                pt = pstr.tile([128, D], MMDT, tag="tr")
                for kd in range(KD):
                    nc.tensor.transpose(
                        pt[:, kd * 128 : (kd + 1) * 128],
                        ypp[:, kd * 128 : (kd + 1) * 128],
                        ident[:],
                    )
                nc.scalar.activation(
                    dest[:, :, dest_off : dest_off + 128],
                    pt[:].rearrange("p (a b) -> p a b", b=128),
                    AF.Copy,
                )

            # ---------------- main pipeline ----------------
            # Software-pipelined across steps (a step = 4 blocks = 512 tokens):
            #   iter i emits: mixD(i) | stageA(i+1) | projMM(i+1) | LN2+transp(i)
            #                 | z-evict(i+1) | FFN1(i) | FFN2(i)
            # so PE always has front-end work of step i+1 while step i's
            # LN chains run on DVE/ACT/GPSIMD.
            zall = {b: [] for b in range(BL)}
            a_out, b_out = {}, {}

            def stage_a(i):
                b, c = steps[i]
                if i not in xpre:
                    preload_x(i)
                xts, xT = xpre.pop(i), []
                stats = [layer_norm_stats(xts[t][:]) for t in range(CB)]
                for t in range(CB):
                    xTt = xtp.tile([128, KD, 128], MMDT, tag="xT")
                    normalize_transpose(xts[t][:], "ypp", xTt, 0, stats=stats[t])
                    xT.append(xTt)
                a_out[i] = (xts, xT)

            def stage_b_mm(i):
                _, xT = a_out[i]
                pzs = []
                for t in range(CB):
                    pz = psmm.tile([128, D], F32, tag="mm")
                    for kd in range(KD):
                        nc.tensor.matmul(
                            pz[:],
                            xT[t][:, kd, :],
                            wp_r[:, kd, :],
                            start=(kd == 0),
                            stop=(kd == KD - 1),
                        )
                    pzs.append(pz)
                b_out[i] = pzs

            def stage_b_evict(i):
                b, c = steps[i]
                for t in range(CB):
                    zt = zp.tile([128, D], MMDT, tag="z")
                    nc.vector.tensor_add(zt[:], b_out[i][t][:], zb_bc[:])
                    zall[b].append(zt)
                del b_out[i]

            stage_a(0)
            stage_b_mm(0)
            stage_b_evict(0)
            # big FFN weights: allocated now, DMA'd in chunks interleaved with
            # the early pipeline so x loads and the first FFN aren't blocked
            # behind 16 MB of weight traffic.
            w1t_r = wgt.tile([128, KD, FF], MMDT, tag="w1t")
            w2t_r = wgt.tile([128, NFT, D], MMDT, tag="w2t")
            w1t_ap = w1t_d.ap().rearrange("(kd p) f -> p kd f", p=128)
            w2t_ap = w2t_d.ap().rearrange("(kf p) d -> p kf d", p=128)
            wload = [
                lambda kd=kd: nc.gpsimd.dma_start(w1t_r[:, kd, :], w1t_ap[:, kd, :])
                for kd in range(KD)
            ] + [
                lambda f4=f4: nc.gpsimd.dma_start(
                    w2t_r[:, 4 * f4 : 4 * f4 + 4, :], w2t_ap[:, 4 * f4 : 4 * f4 + 4, :]
                )
                for f4 in range(4)
            ]
            wload.reverse()  # pop() from the front
            if wload:
                wload.pop()()  # w1t kd=0 immediately
            for i, (b, c) in enumerate(steps):
                xts, _ = a_out.pop(i)
                x2ts, pms = [], []
                # --- mixing (banded); evicts deferred to DVE below ---
                for t in range(CB):
                    blk = c * CB + t
                    nmix = 1 + min(blk, NLAG)
                    pm = psmm.tile([128, D], F32, tag="mm")
                    nc.tensor.matmul(
                        pm[:],
                        wblk_r[:, _BLKIDX[blk], :],
                        zall[b][blk][:],
                        start=True,
                        stop=(nmix == 1),
                    )
                    for l in range(1, nmix):
                        nc.tensor.matmul(
                            pm[:],
                            wlag_r[:, l - 1, :],
                            zall[b][blk - l][:],
                            start=False,
                            stop=(l == nmix - 1),
                        )
                    pms.append(pm)
                # --- prefetch next step stage A (its LN chain runs while the
                # mixing evicts and LN2 chain of this step proceed) ---
                if i + 1 < len(steps):
                    stage_a(i + 1)
                if wload:
                    wload.pop()()
                last = i + 1 == len(steps)
                stats2 = []
                for t in range(CB):
                    x2t = x2p.tile([128, D], F32, tag="x2")
                    nc.vector.tensor_add(x2t[:], pms[t][:], xts[t][:])
                    x2ts.append(x2t)
                    if last:
                        # tail: no next step to hide behind -- start LN2 stats
                        # right after each mixing eviction
                        stats2.append(layer_norm_stats(x2t[:]))
                if i + 1 < len(steps):
                    stage_b_mm(i + 1)
                if wload:
                    wload.pop()()
                # --- LN2 + transpose (per half-chunk dest) ---
                y2T = []
                for hh in range(2):
                    y2Th = y2tp.tile([128, KD, HC], MMDT, tag="y2T")
                    y2T.append(y2Th)
                if not stats2:
                    stats2 = [layer_norm_stats(x2ts[t][:]) for t in range(CB)]
                for t in range(CB):
                    normalize_transpose(
                        x2ts[t][:], "y2pp", y2T[t // 2], (t % 2) * 128,
                        use_dve=True, stats=stats2[t],
                    )
                if wload:
                    wload.pop()()
                if i + 1 < len(steps):
                    stage_b_evict(i + 1)
                # --- FFN1 + FFN2 interleaved per half-chunk ---
                for hh in range(2):
                    ht = hp.tile([128, NFT, HC], MMDT, tag="h")
                    for ft in range(NFT):
                        ph = psmm.tile([128, HC], F32, tag="mm")
                        for kd in range(KD):
                            nc.tensor.matmul(
                                ph[:],
                                w1t_r[:, kd, ft * 128 : (ft + 1) * 128],
                                y2T[hh][:, kd, :],
                                start=(kd == 0),
                                stop=(kd == KD - 1),
                            )
                        nc.scalar.activation(
                            ht[:, ft, :],
                            ph[:],
                            AF.Relu,
                            bias=hb_sb[:, ft : ft + 1],
                            scale=1.0,
                        )
                    for tt in range(2):
                        t = 2 * hh + tt
                        s0 = (c * CB + t) * T
                        po = psmm.tile([128, D], F32, tag="mm")
                        for ft in range(NFT):
                            if wload and ft % 4 == 0:
                                wload.pop()()
                            nc.tensor.matmul(
                                po[:],
                                ht[:, ft, tt * 128 : tt * 128 + 128],
                                w2t_r[:, ft, :],
                                start=(ft == 0),
                                stop=(ft == NFT - 1),
                            )
                        ot = outp.tile([128, D], F32, tag="o")
                        nc.vector.tensor_add(ot[:], po[:], b2_bc[:])
                        nc.vector.tensor_add(ot[:], ot[:], x2ts[t][:])
                        nc.sync.dma_start(out_d.ap()[s0 : s0 + T, b, :], ot[:])

    nc.compile()
    _NC_CACHE[key] = nc
    return nc


def _prep_inputs(x, w_lin, b_lin, w1, b1, w2, b2, g1, beta1, g2, beta2):
    f32 = np.float32
    wp = np.ascontiguousarray(w_lin.T * g1[:, None]).astype(f32)
    zb = (w_lin.astype(np.float64) @ beta1.astype(np.float64) + b_lin).astype(f32)
    w1t = np.ascontiguousarray(w1.T * g2[:, None]).astype(f32)
    hb = (w1.astype(np.float64) @ beta2.astype(np.float64) + b1).astype(f32)
    w2t = np.ascontiguousarray(w2.T).astype(f32)
    shared = {
        "wp": wp,
        "zb": zb,
        "w1t": w1t,
        "hb": hb,
        "w2t": w2t,
        "b2": b2.astype(f32),
        "wblk": _WBLKT,
        "wlag": _WLAGT,
    }
    in_maps = []
    for cc in range(NCORES):
        m = dict(shared)
        m["x"] = np.ascontiguousarray(x[:, cc * BL : (cc + 1) * BL, :]).astype(f32)
        in_maps.append(m)
    return in_maps


def kernel(**inputs):
    nc = build_nc()
    in_maps = _prep_inputs(**inputs)
    res = run_bass_kernel_spmd(nc, in_maps, list(range(NCORES)))
    out = np.concatenate([r["out"] for r in res.results], axis=1)
    return out.astype(np.float32)


if __name__ == "__main__":
    rng = np.random.default_rng(0)
    demo = {
        "x": rng.standard_normal((S, B, D)).astype(np.float32),
        "w_lin": rng.standard_normal((D, D)).astype(np.float32) * D**-0.5,
        "b_lin": rng.standard_normal((D,)).astype(np.float32) * 0.01,
        "w1": rng.standard_normal((FF, D)).astype(np.float32) * D**-0.5,
        "b1": rng.standard_normal((FF,)).astype(np.float32) * 0.01,
        "w2": rng.standard_normal((D, FF)).astype(np.float32) * FF**-0.5,
        "b2": rng.standard_normal((D,)).astype(np.float32) * 0.01,
        "g1": np.ones(D, np.float32),
        "beta1": np.zeros(D, np.float32),
        "g2": np.ones(D, np.float32),
        "beta2": np.zeros(D, np.float32),
    }
    out = kernel(**demo)
    print("ok", out.shape, out.dtype)



# revision 12
# speedup vs baseline: 1.7163x; 1.7163x over previous
"""Trainium2 Bass kernel for nn_ExpSelfAttention (dense transformer block).

Math (per batch item b):
    y  = LN(x; g1, beta1);  z = y @ w_lin.T + b_lin
    attn = W @ z            (W = causal exp-decay matrix, alpha=0.9)
    x2 = x + attn
    y2 = LN(x2; g2, beta2); h = relu(y2 @ w1.T + b1)
    out = x2 + h @ w2.T + b2

Sharding: data parallel over batch (16 / 8 cores = 2 per core); weights and
the (input-independent) decay-matrix blocks replicated. No collectives.

Kernel strategy per core (v2 - mixed precision):
  - Attention path in bf16 (proj + banded mixing matmuls, PE transposes at
    1 cyc/row); residuals and LN stats in f32. b_lin folded into the mixing
    PSUM via a K=1 ones-row bias matmul (W rows sum to 1, so W@(z+zb)=W@z+zb).
  - FFN matmuls in fp8e4 (e4m3) with DoubleRow perf mode: 256 contraction
    rows per instruction at 0.5 cyc per output element - 4x less PE time
    than f32r/bf16. Weights pre-scaled by 16 (w1) / 32 (w2) on the host to
    center fp8's [2^-6, 240] range; the 512x net factor is unwound in the
    output eviction (po * 1/512 + x2b).
  - The S x S decay matmul is block-banded (alpha^128 ~ 1.4e-6): exact
    block-diag + 1 lag matmul per 128-token block.
  - Engine balance: LN normalizes run on the otherwise-idle Pool (gpsimd)
    engine (SBUF-only ops - Pool has no PSUM port); PSUM evictions split
    between ACT and DVE; rsqrt batched 4 tiles per op on ACT (single
    reciprocal_sqrt act table, no reloads).
  - FFN1 PSUM banks hold both 256-token half-chunk groups of one f-tile
    (sequential accumulation groups; PSUM data persists across a group
    start in the same bank), so each relu eviction covers [128,512].
  - Software pipelined in 512-token steps as in v1: step i+1's load/LN1/
    transpose/proj run between step i's mixing and FFN.
"""

import sys
from contextlib import ExitStack

for _p in ("/opt/trn_rl_repo", "/opt/pypackages"):
    if _p not in sys.path:
        sys.path.insert(0, _p)

import numpy as np
import ml_dtypes

import concourse.bass as bass
import concourse.mybir as mybir
import concourse.tile as tile
from concourse import bacc
from concourse.bass_utils import run_bass_kernel_spmd
from concourse.masks import make_identity

ALPHA, EPS = 0.9, 1e-5
S, B, D, FF = 2048, 16, 512, 2048
NCORES = 8
BL = B // NCORES            # batch items per core
T = 128                     # mixing block
CB = 4                      # blocks per chunk (step = 512 tokens)
NBLK = S // T               # 16
NCHUNK = NBLK // CB         # 4
NFT = FF // 128             # 16 f-tiles
KD = D // 128               # 4 d-tiles
NLAG = 1                    # decay lag blocks kept (lag>=2 < 2e-12 relative)
W1SC, W2SC = 16.0, 32.0     # fp8 weight pre-scales
OSC = 1.0 / (W1SC * W2SC)   # output unscale

F32 = mybir.dt.float32
BF16 = mybir.dt.bfloat16
F8 = mybir.dt.float8e4
AF = mybir.ActivationFunctionType
ALU = mybir.AluOpType
DR = mybir.MatmulPerfMode.DoubleRow

NPBF16 = ml_dtypes.bfloat16
NPF8 = mybir.dt.np(F8)      # ml_dtypes.float8_e4m3 (max 240)


def _host_consts():
    """Decay-matrix derived constants, f64 -> f32 (mirrors reference)."""
    i = np.arange(S, dtype=np.float64)
    diff = i[:, None] - i[None, :]
    with np.errstate(under="ignore"):
        W = np.where(diff >= 0, ALPHA ** (diff + 1), 0.0)
        W = W + np.diag(1.0 - W.sum(axis=1))
        W = W.astype(np.float32)
        blocks = [
            np.ascontiguousarray(W[c * T : (c + 1) * T, c * T : (c + 1) * T].T)
            for c in range(NBLK)
        ]
        uniq, idx = [], []
        for blk in blocks:
            for j, u in enumerate(uniq):
                if np.array_equal(blk, u):
                    idx.append(j)
                    break
            else:
                idx.append(len(uniq))
                uniq.append(blk)
        wblkT = np.stack(uniq)  # [NU, T, T]
        lags = []
        for l in range(1, NLAG + 1):
            L = W[l * T : (l + 1) * T, 0:T]
            lags.append(np.ascontiguousarray(L.T))
        wlagT = np.stack(lags)  # [NLAG, T, T]
    return wblkT.astype(np.float32), idx, wlagT.astype(np.float32)


_WBLKT, _BLKIDX, _WLAGT = _host_consts()
NU = _WBLKT.shape[0]

_NC_CACHE = {}


def build_nc():
    key = "v2"
    if key in _NC_CACHE:
        return _NC_CACHE[key]
    nc = bacc.Bacc()

    x_d = nc.declare_dram_parameter("x", [S, BL, D], F32, isOutput=False)
    wp_d = nc.declare_dram_parameter("wp", [D, D], BF16, isOutput=False)
    zb_d = nc.declare_dram_parameter("zb", [D], BF16, isOutput=False)
    w1_d = nc.declare_dram_parameter("w1t8", [D, FF], F8, isOutput=False)
    hb_d = nc.declare_dram_parameter("hb16", [FF], F32, isOutput=False)
    w2_d = nc.declare_dram_parameter("w2t8", [FF, D], F8, isOutput=False)
    b2_d = nc.declare_dram_parameter("b2", [D], F32, isOutput=False)
    wblk_d = nc.declare_dram_parameter("wblk", [NU, T, T], BF16, isOutput=False)
    wlag_d = nc.declare_dram_parameter("wlag", [NLAG, T, T], BF16, isOutput=False)
    out_d = nc.declare_dram_parameter("out", [S, BL, D], F32, isOutput=True)

    with tile.TileContext(nc) as tc, ExitStack() as ctx:
        pool = lambda name, bufs, **kw: ctx.enter_context(
            tc.tile_pool(name=name, bufs=bufs, **kw)
        )
        wgt = pool("wgt", 1)
        stage = pool("stage", 1)
        xin = pool("xin", 9)
        lnp = pool("ln", 4)
        yppp = pool("ypp", 2)
        xtp = pool("xt", 6)
        y2tp = pool("y2t", 2)
        zp = pool("z", 10)
        x2p = pool("x2", 5)
        x2bp = pool("x2b", 5)
        hp = pool("h", 2)
        outp = pool("outp", 3)
        psmm = pool("psmm", 5, space="PSUM")
        pstr = pool("pstr", 3, space="PSUM")

        # ---------------- one-time setup ----------------
        xpre = {}

        def preload_x(i):
            b, c = steps[i]
            tiles = []
            for t in range(CB):
                s0 = (c * CB + t) * T
                xt = xin.tile([128, D], F32, tag="x")
                nc.sync.dma_start(xt[:], x_d.ap()[s0 : s0 + T, b, :])
                tiles.append(xt)
            xpre[i] = tiles

        steps = [(b, c) for b in range(BL) for c in range(NCHUNK)]
        preload_x(0)

        ident_f = stage.tile([128, 128], F32, tag="ident_f")
        make_identity(nc, ident_f[:])
        ident_bf = wgt.tile([128, 128], BF16, tag="ident_bf")
        nc.vector.tensor_copy(ident_bf[:], ident_f[:])
        ident_f8 = wgt.tile([128, 128], F8, tag="ident_f8")
        nc.vector.tensor_copy(ident_f8[:], ident_f[:])
        eps_t = wgt.tile([128, 1], F32, tag="eps")
        nc.vector.memset(eps_t[:], EPS)
        ones_r = wgt.tile([1, 128], BF16, tag="ones_r")
        nc.vector.memset(ones_r[:], 1.0)
        zb_r = wgt.tile([1, D], BF16, tag="zb_r")
        nc.sync.dma_start(zb_r[:], bass.AP(tensor=zb_d, offset=0, ap=[[0, 1], [1, D]]))
        b2_bc = wgt.tile([128, D], F32, tag="b2")
        nc.sync.dma_start(
            b2_bc[:], bass.AP(tensor=b2_d, offset=0, ap=[[0, 128], [1, D]])
        )
        hb_sb = wgt.tile([128, NFT], F32, tag="hb")
        nc.sync.dma_start(
            hb_sb[:], bass.AP(tensor=hb_d, offset=0, ap=[[1, 128], [128, NFT]])
        )
        # attention-path weights (small): load now
        wp_r = wgt.tile([128, KD, D], BF16, tag="wp")
        nc.sync.dma_start(wp_r[:], wp_d.ap().rearrange("(kd p) e -> p kd e", p=128))
        wblk_r = wgt.tile([128, NU, T], BF16, tag="wblk")
        nc.sync.dma_start(wblk_r[:], wblk_d.ap().rearrange("b j r -> j b r"))
        wlag_r = wgt.tile([128, NLAG, T], BF16, tag="wlag")
        nc.sync.dma_start(wlag_r[:], wlag_d.ap().rearrange("b j r -> j b r"))

        # ---------------- helpers ----------------
        def ln_stats(xts, tag):
            """4 tiles' LN stats -> (mvs [128,4,2], rstd4 [128,4])."""
            mvs = lnp.tile([128, CB, 2], F32, tag=tag + "mv")
            for t in range(CB):
                st = lnp.tile([128, 6], F32, tag=tag + "st")
                nc.vector.bn_stats(st[:], xts[t][:])
                nc.vector.bn_aggr(mvs[:, t, :], st[:])
            rstd4 = lnp.tile([128, CB], F32, tag=tag + "rs")
            nc.scalar.activation(
                rstd4[:],
                mvs[:, :, 1:2].rearrange("p a b -> p (a b)"),
                AF.Sqrt, bias=eps_t[:], scale=1.0,
            )
            nc.vector.reciprocal(rstd4[:], rstd4[:])
            return mvs, rstd4

        def norm_transp(xt, mvs, rstd4, t, odt, ident, dest_ap, src_pat, tag):
            """(xt - m)*rstd -> odt tile, PE-transpose, ACT-evict to dest."""
            ypp = yppp.tile([128, D], odt, tag=tag)
            nc.gpsimd.tensor_scalar(
                out=ypp[:], in0=xt[:],
                scalar1=mvs[:, t, 0:1], scalar2=rstd4[:, t : t + 1],
                op0=ALU.subtract, op1=ALU.mult,
            )
            pt = pstr.tile([128, 512], odt, tag="tr")
            for kd in range(KD):
                nc.tensor.transpose(
                    pt[:, kd * 128 : (kd + 1) * 128],
                    ypp[:, kd * 128 : (kd + 1) * 128],
                    ident[:],
                )
            nc.scalar.activation(dest_ap, src_pat(pt[:]), AF.Copy)

        # ---------------- pipeline stages ----------------
        zall = {b: [] for b in range(BL)}
        a_out, b_out = {}, {}

        def stage_a(i):
            b, c = steps[i]
            if i not in xpre:
                preload_x(i)
            xts, xT = xpre.pop(i), []
            mvs, rstd4 = ln_stats(xts, "l1")
            for t in range(CB):
                xTt = xtp.tile([128, KD, 128], BF16, tag="xT")
                norm_transp(xts[t], mvs, rstd4, t, BF16, ident_bf,
                            xTt[:],
                            lambda p: p.rearrange("p (a b) -> p a b", b=128), "y1")
                xT.append(xTt)
            a_out[i] = (xts, xT)

        def stage_b_mm(i):
            _, xT = a_out[i]
            pzs = []
            for t in range(CB):
                pz = psmm.tile([128, D], F32, tag="mm")
                for kd in range(KD):
                    nc.tensor.matmul(
                        pz[:], xT[t][:, kd, :], wp_r[:, kd, :],
                        start=(kd == 0), stop=(kd == KD - 1),
                    )
                pzs.append(pz)
            b_out[i] = pzs

        def stage_b_evict(i):
            b, c = steps[i]
            for t in range(CB):
                zt = zp.tile([128, D], BF16, tag="z")
                nc.scalar.activation(zt[:], b_out[i][t][:], AF.Copy)
                zall[b].append(zt)
            del b_out[i]

        stage_a(0)
        stage_b_mm(0)
        stage_b_evict(0)

        # big fp8 FFN weights: DMA'd in chunks interleaved with the early
        # pipeline so x loads and the first FFN aren't blocked.
        w18_r = wgt.tile([128, 2, 2, FF], F8, tag="w18")
        w28_r = wgt.tile([128, NFT // 2, 2, D], F8, tag="w28")
        w18_ap = w1_d.ap().rearrange("(kd2 i p) f -> p kd2 i f", p=128, i=2)
        w28_ap = w2_d.ap().rearrange("(kt i p) e -> p kt i e", p=128, i=2)
        wload = [
            lambda kd2=kd2: nc.sync.dma_start(
                w18_r[:, kd2, :, :], w18_ap[:, kd2, :, :]
            )
            for kd2 in range(2)
        ] + [
            lambda k4=k4: nc.sync.dma_start(
                w28_r[:, 2 * k4 : 2 * k4 + 2, :, :], w28_ap[:, 2 * k4 : 2 * k4 + 2, :, :]
            )
            for k4 in range(4)
        ]
        wload.reverse()
        if wload:
            wload.pop()()

        for i, (b, c) in enumerate(steps):
            xts, _ = a_out.pop(i)
            x2ts, x2bts, pms = [], [], []
            # --- mixing (banded) + zb ones-row bias matmul ---
            for t in range(CB):
                blk = c * CB + t
                nmix = 1 + min(blk, NLAG)
                pm = psmm.tile([128, D], F32, tag="mm")
                nc.tensor.matmul(
                    pm[:], wblk_r[:, _BLKIDX[blk], :], zall[b][blk][:],
                    start=True, stop=False,
                )
                for l in range(1, nmix):
                    nc.tensor.matmul(
                        pm[:], wlag_r[:, l - 1, :], zall[b][blk - l][:],
                        start=False, stop=False,
                    )
                nc.tensor.matmul(pm[:], ones_r[:], zb_r[:], start=False, stop=True)
                pms.append(pm)
            # --- prefetch next step's stage A ---
            if i + 1 < len(steps):
                stage_a(i + 1)
            if wload:
                wload.pop()()
            # --- x2 = pm + x (DVE, f32); x2b = x2 + b2 (Pool) ---
            for t in range(CB):
                x2t = x2p.tile([128, D], F32, tag="x2")
                nc.vector.tensor_add(x2t[:], pms[t][:], xts[t][:])
                x2ts.append(x2t)
                x2bt = x2bp.tile([128, D], F32, tag="x2b")
                nc.gpsimd.tensor_add(x2bt[:], x2t[:], b2_bc[:])
                x2bts.append(x2bt)
            if i + 1 < len(steps):
                stage_b_mm(i + 1)
            if wload:
                wload.pop()()
            # --- LN2 + transpose into y2T [128, kd2, i, 512] fp8 ---
            y2T = y2tp.tile([128, 2, 2, 512], F8, tag="y2T")
            mvs2, rstd42 = ln_stats(x2ts, "l2")
            for t in range(CB):
                norm_transp(
                    x2ts[t], mvs2, rstd42, t, F8, ident_f8,
                    y2T[:, :, :, t * 128 : (t + 1) * 128],
                    lambda p: p.rearrange("p (a c b) -> p a c b", a=2, c=2, b=128),
                    "y2",
                )
            if wload:
                wload.pop()()
            if i + 1 < len(steps):
                stage_b_evict(i + 1)
            # --- FFN1: fp8 DoubleRow, one PSUM bank per f-tile (two
            #     256-token groups), relu evict alternating ACT/DVE ---
            h8 = hp.tile([128, NFT // 2, 2, 512], F8, tag="h")
            for ft in range(NFT):
                ph = psmm.tile([128, 512], F32, tag="mm")
                for hh in range(2):
                    for kd2 in range(2):
                        nc.tensor.matmul(
                            ph[:, hh * 256 : (hh + 1) * 256],
                            w18_r[:, kd2, :, ft * 128 : (ft + 1) * 128],
                            y2T[:, kd2, :, hh * 256 : (hh + 1) * 256],
                            start=(kd2 == 0), stop=(kd2 == 1),
                            perf_mode=DR,
                        )
                hdst = h8[:, ft // 2, ft % 2, :]
                hbc = hb_sb[:, ft : ft + 1]
                if ft % 2 == 0:
                    nc.scalar.activation(hdst, ph[:], AF.Relu, bias=hbc, scale=1.0)
                else:
                    nc.vector.tensor_scalar(
                        out=hdst, in0=ph[:], scalar1=hbc, scalar2=0.0,
                        op0=ALU.add, op1=ALU.max,
                    )
                if wload and ft % 6 == 5:
                    wload.pop()()
            # --- FFN2: fp8 DoubleRow, two 256-col groups per out tile ---
            for t in range(CB):
                s0 = (c * CB + t) * T
                po = psmm.tile([128, D], F32, tag="mm")
                for eh in range(2):
                    for kt in range(NFT // 2):
                        nc.tensor.matmul(
                            po[:, eh * 256 : (eh + 1) * 256],
                            h8[:, kt, :, t * 128 : (t + 1) * 128],
                            w28_r[:, kt, :, eh * 256 : (eh + 1) * 256],
                            start=(kt == 0), stop=(kt == NFT // 2 - 1),
                            perf_mode=DR,
                        )
                ot = outp.tile([128, D], F32, tag="o")
                nc.vector.scalar_tensor_tensor(
                    out=ot[:], in0=po[:], scalar=OSC, in1=x2bts[t][:],
                    op0=ALU.mult, op1=ALU.add,
                )
                nc.sync.dma_start(out_d.ap()[s0 : s0 + T, b, :], ot[:])

    nc.compile()
    _NC_CACHE[key] = nc
    return nc


def _prep_inputs(x, w_lin, b_lin, w1, b1, w2, b2, g1, beta1, g2, beta2):
    f32 = np.float32
    wp = np.ascontiguousarray(w_lin.T * g1[:, None]).astype(NPBF16)
    zb = (w_lin.astype(np.float64) @ beta1.astype(np.float64) + b_lin).astype(
        f32
    ).astype(NPBF16)
    w1t8 = np.ascontiguousarray(W1SC * w1.T * g2[:, None]).astype(NPF8)
    hb16 = (W1SC * (w1.astype(np.float64) @ beta2.astype(np.float64) + b1)).astype(f32)
    w2t8 = np.ascontiguousarray(W2SC * w2.T).astype(NPF8)
    shared = {
        "wp": wp,
        "zb": zb,
        "w1t8": w1t8,
        "hb16": hb16,
        "w2t8": w2t8,
        "b2": b2.astype(f32),
        "wblk": _WBLKT.astype(NPBF16),
        "wlag": _WLAGT.astype(NPBF16),
    }
    in_maps = []
    for cc in range(NCORES):
        m = dict(shared)
        m["x"] = np.ascontiguousarray(x[:, cc * BL : (cc + 1) * BL, :]).astype(f32)
        in_maps.append(m)
    return in_maps


def kernel(**inputs):
    nc = build_nc()
    in_maps = _prep_inputs(**inputs)
    res = run_bass_kernel_spmd(nc, in_maps, list(range(NCORES)))
    out = np.concatenate([r["out"] for r in res.results], axis=1)
    return out.astype(np.float32)


if __name__ == "__main__":
    rng = np.random.default_rng(0)
    demo = {
        "x": rng.standard_normal((S, B, D)).astype(np.float32),
        "w_lin": rng.standard_normal((D, D)).astype(np.float32) * D**-0.5,
        "b_lin": rng.standard_normal((D,)).astype(np.float32) * 0.01,
        "w1": rng.standard_normal((FF, D)).astype(np.float32) * D**-0.5,
        "b1": rng.standard_normal((FF,)).astype(np.float32) * 0.01,
        "w2": rng.standard_normal((D, FF)).astype(np.float32) * FF**-0.5,
        "b2": rng.standard_normal((D,)).astype(np.float32) * 0.01,
        "g1": np.ones(D, np.float32),
        "beta1": np.zeros(D, np.float32),
        "g2": np.ones(D, np.float32),
        "beta2": np.zeros(D, np.float32),
    }
    out = kernel(**demo)
    print("ok", out.shape, out.dtype)
